# revision 9
# baseline (speedup 1.0000x reference)
"""MiniMHSA Trainium2 kernel: 8 NeuronCores, shard = (batch n, head-group).

Reference, per batch n:
  qkv = x @ W_qkv.T + b_qkv ; heads (H=16, HD=64)
  scores = (q @ k.T)/sqrt(HD), masked keys -> -1e9, softmax, @ v
  out = attn_out @ W_out.T + b_out

Core c handles n = c//2, head-group hg = c%2 (HC=8 heads). Key ideas:

* Mask compaction: masked keys contribute exp(-1e9)=0 exactly, so the host
  gathers only the ~L/2 unmasked key columns (zero-padded to KP, a multiple
  of 128). k/v projections, scores, exp and attn@v run only over KP keys —
  this halves both the PE attention work and the ACT exp work.
* All-bf16 datapath (fp8 fails the 2e-2 gate: quantizing q/k tilts every
  score of a row coherently, and sharp softmax rows pass v/p noise through
  unaveraged). PSUM accumulation is fp32.
* Bias algebra: the k-projection bias shifts all scores of a query row
  equally -> cancels in softmax (dropped). The v bias shifts attn output by
  const bv -> host adds (W_out @ bv + b_out) once. Only the q bias survives
  on-device, folded into the psum->bf16 convert.
* Softmax denominator: vp carries a 65th column of ones, so attn@v
  accumulates [sum(p*v); sum(p)] in one pass; normalize = DVE reciprocal +
  PE ones-broadcast + DVE multiply. No max-subtraction (scores <= ~10, exp
  in fp32 psum is safe).
* No DRAM bounce: otn lives in SBUF, head pairs partition-stacked (K=128
  out-projection); odd heads hop partitions 0-63 -> 64-127 via a tiny
  SBUF->SBUF DMA. Out-projection of hemisphere i overlaps attention of
  hemisphere i+1. Host sums the two head-group partial y's (bf16).
"""
import sys

sys.path.insert(0, '/opt/trn_rl_repo')

import numpy as np
import ml_dtypes

BF16NP = ml_dtypes.bfloat16

_KERNEL_CACHE = {}
_NEG = -1e9


def _split_excess_waits(nc):
    """Walrus codegen reliably accepts only ONE sync wait per instruction.
    Move excess waits onto preceding same-engine NOPs (engine queues are
    in-order, so this is semantically identical)."""
    from concourse import mybir

    for f in nc.m.functions:
        for blk in f.blocks:
            il = blk.instructions
            i = 0
            while i < len(il):
                inst = il[i]
                si = inst.sync_info
                waits = list(si.on_wait) if si is not None and si.on_wait else []
                if len(waits) > 1:
                    keep = waits[-1:]
                    excess = waits[:-1]
                    pos = i
                    for j, wcond in enumerate(excess):
                        nop = mybir.InstNoOp(name=f"{inst.name}-ws{j}", ins=[], outs=[])
                        nop.engine = inst.engine
                        nop.sync_info = mybir.SyncInfo(on_wait=[wcond], on_update=[])
                        il.insert(pos, nop)
                        pos += 1
                        i += 1
                    inst.sync_info = mybir.SyncInfo(
                        on_wait=keep,
                        on_update=list(si.on_update) if si.on_update else [],
                    )
                i += 1


def _build(cfg, waitsplit=True):
    import concourse.bass as bass
    import concourse.tile as tile
    from concourse import mybir

    F32 = mybir.dt.float32
    F32R = mybir.dt.float32r
    BF16 = mybir.dt.bfloat16
    AF = mybir.ActivationFunctionType
    MULT = mybir.AluOpType.mult

    L, D, HC, HD, KP = cfg["L"], cfg["D"], cfg["HC"], cfg["HD"], cfg["KP"]
    DCH = D // 128            # contraction chunks (8)
    DV = HC * HD              # per-core qkv width (512)
    MQ = DV // 128            # q/k M-chunks (4)
    KC = KP // 128            # key chunks
    HEMI = 1024
    NH = L // HEMI            # 2
    DC = D // 512             # 2
    VPW = HD + 2              # vp row: 64 v cols + ones col + pad

    nc = bass.Bass()
    xT_d = nc.dram_tensor("xT", [D, L], BF16, kind="ExternalInput")
    xk_d = nc.dram_tensor("xk", [D, KP], BF16, kind="ExternalInput")
    wq_d = nc.dram_tensor("wq", [128, DCH, DV], BF16, kind="ExternalInput")
    wk_d = nc.dram_tensor("wk", [128, DCH, DV], BF16, kind="ExternalInput")
    wv_d = nc.dram_tensor("wv", [128, DCH, DV], BF16, kind="ExternalInput")
    wo_d = nc.dram_tensor("wo", [128, MQ, D], BF16, kind="ExternalInput")
    bq_d = nc.dram_tensor("bq", [128, MQ], F32, kind="ExternalInput")
    mb_d = nc.dram_tensor("mb", [128, KC], F32, kind="ExternalInput")
    y_d = nc.dram_tensor("y", [L, D], BF16, kind="ExternalOutput")

    with tile.TileContext(nc) as tc, \
         nc.allow_low_precision(reason="bf16 matmuls intended"):
        with tc.tile_pool(name="const", bufs=1) as const, \
             tc.tile_pool(name="big", bufs=1) as big, \
             tc.tile_pool(name="workP", bufs=4) as workP, \
             tc.tile_pool(name="workS", bufs=3) as workS, \
             tc.tile_pool(name="workY", bufs=2) as workY:

            # ---- constants / weights ----
            bq_t = const.tile([128, MQ], F32, tag="bq")
            nc.sync.dma_start(out=bq_t, in_=bq_d[:, :])
            mb_t = const.tile([128, KC], F32, tag="mb")
            nc.sync.dma_start(out=mb_t, in_=mb_d[:, :])
            ones_f = const.tile([128, 1], F32, tag="onesf")
            nc.vector.memset(ones_f, 1.0)
            ones_r = const.tile([1, 64], F32R, tag="onesr")
            nc.vector.tensor_copy(
                out=ones_r, in_=ones_f[0:1, 0:1].broadcast_to([1, 64]))

            wk_t = big.tile([128, DCH, DV], BF16, tag="wk")
            nc.gpsimd.dma_start(out=wk_t, in_=wk_d[:, :, :])
            xk_t = big.tile([128, DCH, KP], BF16, tag="xk")
            xk_r = xk_d.rearrange("(c p) l -> p c l", p=128)
            half = KP // 2
            nc.gpsimd.dma_start(out=xk_t[:, :, 0:half], in_=xk_r[:, :, 0:half])
            nc.sync.dma_start(out=xk_t[:, :, half:KP], in_=xk_r[:, :, half:KP])
            wv_t = big.tile([128, DCH, DV], BF16, tag="wv")
            nc.gpsimd.dma_start(out=wv_t, in_=wv_d[:, :, :])
            wq_t = big.tile([128, DCH, DV], BF16, tag="wq")
            nc.gpsimd.dma_start(out=wq_t, in_=wq_d[:, :, :])
            xT_t = big.tile([128, DCH, L], BF16, tag="xT")
            nc.sync.dma_start(
                out=xT_t, in_=xT_d.rearrange("(c p) l -> p c l", p=128))
            wo_t = big.tile([128, MQ, D], BF16, tag="wo")
            nc.sync.dma_start(out=wo_t, in_=wo_d[:, :, :])

            qT_t = big.tile([128, MQ, L], BF16, tag="qT")
            kT_t = big.tile([128, MQ, KP], BF16, tag="kT")
            vp_t = big.tile([128, KC, HC, VPW], BF16, tag="vp")
            nc.vector.memset(vp_t[:, :, :, HD:HD + 1], 1.0)  # denom column
            otn_t = big.tile([128, MQ, L], BF16, tag="otn")

            # ---- k/v projections + q (hemi 0) ----
            with tc.tile_pool(name="psA", bufs=2, space="PSUM") as psA:
                # k^T: [dv, KP] in 4 M-chunks; head pair per chunk
                for m in range(MQ):
                    for c0 in range(0, KP, 512):
                        w = min(512, KP - c0)
                        ps = psA.tile([128, 512], F32, tag="p")
                        for t in range(DCH):
                            nc.tensor.matmul(
                                ps[:, 0:w],
                                wk_t[:, t, m * 128:(m + 1) * 128],
                                xk_t[:, t, c0:c0 + w],
                                start=(t == 0), stop=(t == DCH - 1))
                        nc.vector.tensor_copy(
                            out=kT_t[:, m, c0:c0 + w], in_=ps[:, 0:w])
                # v: [kpos, dv] natural; per key chunk
                for kc in range(KC):
                    ps = psA.tile([128, 512], F32, tag="p")
                    for t in range(DCH):
                        nc.tensor.matmul(
                            ps,
                            xk_t[:, t, kc * 128:(kc + 1) * 128],
                            wv_t[:, t, :],
                            start=(t == 0), stop=(t == DCH - 1))
                    nc.vector.tensor_copy(
                        out=vp_t[:, kc, :, 0:HD],
                        in_=ps.rearrange("p (h d) -> p h d", h=HC))
                # q^T for hemisphere 0 (hemi 1 is emitted as attention filler)
                for lc in range(HEMI // 512):
                    for m in range(MQ):
                        ps = psA.tile([128, 512], F32, tag="p")
                        for t in range(DCH):
                            nc.tensor.matmul(
                                ps,
                                wq_t[:, t, m * 128:(m + 1) * 128],
                                xT_t[:, t, lc * 512:(lc + 1) * 512],
                                start=(t == 0), stop=(t == DCH - 1))
                        nc.vector.tensor_scalar_add(
                            out=qT_t[:, m, lc * 512:(lc + 1) * 512],
                            in0=ps, scalar1=bq_t[:, m:m + 1])

            # ---- attention, with q-proj / out-proj tiles as PE fillers ----
            # psB is a shared 3-slot ring (4KB slots): scores (st), the
            # ones-broadcast (bc), q-proj and out-proj psums. psC holds the
            # long-lived attn@v accumulator.
            with tc.tile_pool(name="psB", bufs=2, space="PSUM") as psB, \
                 tc.tile_pool(name="psC", bufs=1, space="PSUM") as psC, \
                 tc.tile_pool(name="psF", bufs=2, space="PSUM") as psF:

                def qproj_tile(lc, m):
                    ps = psF.tile([128, 512], F32, tag="fill")
                    for t in range(DCH):
                        nc.tensor.matmul(
                            ps,
                            wq_t[:, t, m * 128:(m + 1) * 128],
                            xT_t[:, t, lc * 512:(lc + 1) * 512],
                            start=(t == 0), stop=(t == DCH - 1))
                    nc.vector.tensor_scalar_add(
                        out=qT_t[:, m, lc * 512:(lc + 1) * 512],
                        in0=ps, scalar1=bq_t[:, m:m + 1])

                def outproj_tile(qr):
                    y_sb = workY.tile([128, D], BF16, tag="y")
                    for dc in range(DC):
                        y_ps = psF.tile([128, 512], F32, tag="fill")
                        for i in range(MQ):
                            nc.tensor.matmul(
                                y_ps,
                                otn_t[:, i, qr:qr + 128],
                                wo_t[:, i, dc * 512:(dc + 1) * 512],
                                start=(i == 0), stop=(i == MQ - 1))
                        nc.vector.tensor_copy(
                            out=y_sb[:, dc * 512:(dc + 1) * 512], in_=y_ps)
                    nc.sync.dma_start(out=y_d[qr:qr + 128, :], in_=y_sb)

                fillers = [(qproj_tile, (lc, m))
                           for lc in range(HEMI // 512, L // 512)
                           for m in range(MQ)]

                for hemi in range(NH):
                    q0 = hemi * HEMI
                    for h in range(HC):
                        m, b = h // 2, 64 * (h % 2)
                        ot = psC.tile([HD + 1, HEMI], F32, tag="ot")

                        def scores(kc):
                            st = psB.tile([128, HEMI], F32, tag="st")
                            for s in range(2):
                                nc.tensor.matmul(
                                    st[:, s * 512:(s + 1) * 512],
                                    kT_t[b:b + 64, m, kc * 128:(kc + 1) * 128],
                                    qT_t[b:b + 64, m, q0 + s * 512:q0 + (s + 1) * 512],
                                    start=True, stop=True)
                            pT = workP.tile([128, HEMI], BF16, tag="pT")
                            nc.scalar.activation(
                                out=pT, in_=st, func=AF.Exp,
                                bias=mb_t[:, kc:kc + 1], scale=0.125)
                            return pT

                        def attnv(kc, pT):
                            for s in range(2):
                                nc.tensor.matmul(
                                    ot[:, s * 512:(s + 1) * 512],
                                    vp_t[:, kc, h, 0:HD + 1],
                                    pT[:, s * 512:(s + 1) * 512],
                                    start=(kc == 0), stop=(kc == KC - 1))

                        # software pipeline: scores run one chunk ahead of
                        # attn@v so the in-order PE queue never heads-blocks
                        # on the exp.
                        pT_prev = scores(0)
                        for kc in range(1, KC):
                            pT_cur = scores(kc)
                            attnv(kc - 1, pT_prev)
                            pT_prev = pT_cur
                            if kc == KC // 2 and fillers:
                                fn, args = fillers.pop(0)
                                fn(*args)
                        attnv(KC - 1, pT_prev)

                        # normalize: otn = ot[0:64] * (1/ot[64]); the
                        # broadcast rides a stride-0 SBUF->SBUF DMA so the
                        # PE and DVE stay out of it.
                        recip = workS.tile([1, HEMI], F32, tag="recip")
                        nc.vector.reciprocal(out=recip, in_=ot[HD:HD + 1, :])
                        bc_sb = workS.tile([64, HEMI], F32, tag="bc")
                        nc.sync.dma_start(
                            out=bc_sb,
                            in_=recip.unsqueeze(1).broadcast_to([1, 64, HEMI]))
                        # fill the PE's ACT-bound gap while the broadcast
                        # lands; the next head's attn@v waits on the multiply
                        # (psC single slot), so this hides that latency too.
                        if fillers:
                            fn, args = fillers.pop(0)
                            fn(*args)
                        if h % 2 == 0:
                            nc.vector.tensor_tensor(
                                out=otn_t[0:64, m, q0:q0 + HEMI],
                                in0=ot[0:HD, :], in1=bc_sb, op=MULT)
                        else:
                            tmp = workS.tile([64, HEMI], BF16, tag="tmp")
                            nc.vector.tensor_tensor(
                                out=tmp, in0=ot[0:HD, :], in1=bc_sb, op=MULT)
                            nc.sync.dma_start(
                                out=otn_t[64:128, m, q0:q0 + HEMI], in_=tmp)
                    # out-projection tiles become fillers for the next
                    # hemisphere's attention; the final hemisphere drains
                    # right here.
                    fillers += [(outproj_tile, (q0 + qt * 128,))
                                for qt in range(HEMI // 128)]
                for fn, args in fillers:
                    fn(*args)

    if waitsplit:
        _split_excess_waits(nc)
    return nc


def _prep_inputs(x, mask, W_qkv, b_qkv, W_out, b_out, cfg):
    """Build the 8 per-core input maps (host-side shuffles + bf16 casts)."""
    L, D, HC, HD, KP = cfg["L"], cfg["D"], cfg["HC"], cfg["HD"], cfg["KP"]
    DV = HC * HD
    N = x.shape[0]
    DCH = D // 128
    MQ = DV // 128
    KC = KP // 128

    Wt = np.ascontiguousarray(W_qkv.T).astype(np.float32)    # [D, 3D]
    WoT = np.ascontiguousarray(W_out.T).astype(np.float32)   # [D, D]

    def chunked(w):  # [D, cols] -> [128, DCH, cols]
        return np.ascontiguousarray(
            w.reshape(DCH, 128, w.shape[1]).transpose(1, 0, 2)).astype(BF16NP)

    per_hg = []
    for hg in range(2):
        qs, ks, vs = hg * DV, D + hg * DV, 2 * D + hg * DV
        wq = chunked(Wt[:, qs:qs + DV])
        wk = chunked(Wt[:, ks:ks + DV])
        wv = chunked(Wt[:, vs:vs + DV])
        # wo: head-pair partition stacking = natural row order [128, MQ, D]
        wo = np.ascontiguousarray(
            WoT[hg * DV:(hg + 1) * DV, :].reshape(MQ, 128, D)
            .transpose(1, 0, 2)).astype(BF16NP)
        bq = np.ascontiguousarray(
            b_qkv[qs:qs + DV].reshape(MQ, 128).T).astype(np.float32)
        per_hg.append(dict(wq=wq, wk=wk, wv=wv, wo=wo, bq=bq))

    per_n = []
    for n in range(N):
        kept = np.flatnonzero(~mask[n])
        xT = np.ascontiguousarray(x[n].T).astype(BF16NP)
        xk = np.zeros((D, KP), np.float32)
        xk[:, :len(kept)] = x[n][kept].T
        xk = xk.astype(BF16NP)
        mb = np.full(KP, _NEG, np.float32)
        mb[:len(kept)] = 0.0
        mb = np.ascontiguousarray(mb.reshape(KC, 128).T)
        per_n.append(dict(xT=xT, xk=xk, mb=mb))

    in_maps = []
    for c in range(2 * N):
        n, hg = c // 2, c % 2
        d = dict(per_hg[hg])
        d.update(per_n[n])
        in_maps.append(d)
    return in_maps


def kernel(x, mask, W_qkv, b_qkv, W_out, b_out):
    from concourse.bass_utils import run_bass_kernel_spmd

    x = np.asarray(x, dtype=np.float32)
    mask = np.asarray(mask).astype(bool)
    N, L, D = x.shape
    H = 16
    HD = D // H

    W_qkv = np.asarray(W_qkv, np.float32)
    b_qkv = np.asarray(b_qkv, np.float32)
    W_out = np.asarray(W_out, np.float32)
    b_out = np.asarray(b_out, np.float32)

    # Per-batch key padding: cores of a batch run a kernel sized to that
    # batch's unmasked-key count (builds are cached per KP).
    kepts = (~mask).sum(axis=1)
    KPs = [max(256, ((int(k) + 127) // 128) * 128) for k in kepts]
    ys = [None] * (2 * N)
    for KP in sorted(set(KPs)):
        batches = [n for n in range(N) if KPs[n] == KP]
        cfg = {"L": L, "D": D, "HC": H // 2, "HD": HD, "KP": KP}
        key = (L, D, H, KP)
        if key not in _KERNEL_CACHE:
            _KERNEL_CACHE[key] = _build(cfg)
        nc = _KERNEL_CACHE[key]
        sub_x = x[batches]
        sub_mask = mask[batches]
        in_maps = _prep_inputs(
            sub_x, sub_mask, W_qkv, b_qkv, W_out, b_out, cfg)
        res = run_bass_kernel_spmd(nc, in_maps, list(range(2 * len(batches))))
        for i, n in enumerate(batches):
            ys[2 * n] = np.asarray(res.results[2 * i]["y"], dtype=np.float32)
            ys[2 * n + 1] = np.asarray(
                res.results[2 * i + 1]["y"], dtype=np.float32)

    # v-bias shifts attn output by const bv: y += W_out @ bv (+ b_out)
    extra = (W_out @ b_qkv[2 * D:3 * D] + b_out).astype(np.float32)
    out = np.empty((N, L, D), np.float32)
    for n in range(N):
        out[n] = ys[2 * n] + ys[2 * n + 1] + extra
    return out
